# revision 25
# baseline (speedup 1.0000x reference)
"""Trainium2 Bass kernel for nn_AttentionHead_Hybrid2 (B=4, N=4096, DK=64).

reference:
    V = x @ Wv.T + bv              (B,N,DK)
    Q = x @ wq ; K = x @ wk        (B,N)
    A = exp(-(Q_i - K_j)^2)        (B,N,N)
    P = softmax(A / 8, axis=-1)
    out = LN(P @ V + x)

Sharding: 8 cores = (batch b = c//2) x (query half c%2). Each core gets the
full key/value set for its batch (rolled so its 2048 queries are rows 0:2048)
and produces its 2048x64 output slice.

Key idea: the score between query i and key j depends on j ONLY through the
scalar K_j. Keys are binned onto a uniform 512-point grid over K-space with
linear (hat-function) interpolation, which is exact to O(delta^2):
    e(Q_i, K_j) ~= sum_m w_jm * e(Q_i, kappa_m),   w = hat((K_j-kappa_m)/delta)
so
    out_i = sum_j e_ij Vaug_j = sum_m E(Q_i, kappa_m) * (sum_j w_jm Vaug_j)
collapsing the (2048 x 4096) score work to (2048 x 512) plus cheap binning.
The hat errors are oscillatory in j and average out; measured end-to-end
error stays at fp32 noise (~1e-6 relative).

Per-core phases:
    prep:   xTb = [x.T ; 1] via DMA (host supplies x.T); [V+bv | K] = xTb.T @
            [Wv.T|wk ; bv|0]; Q row; q_rep via DRAM-broadcast DMA
    bin:    per key-tile jt: t = -clamp((K - k0)/d);  u = |m + t|  (ACT Abs)
            w = relu(1 - u)  (DVE + GpSimd);  bvt[65, m] += Vaug_jt.T @ W (PE)
    score:  per bin chunk (128 bins): E = exp(exp(-(kappa-Q)^2)/8) (3 ACT
            passes, interleaved with bin phase); accT[65,i] += bva_mc.T @ E
    finish: transpose accT, divide by rowsum (col 64), + x, LayerNorm, DMA.
"""

import sys

for _p in ("/opt/trn_rl_repo", "/root/.axon_site/_ro/trn_rl_repo"):
    if _p not in sys.path:
        sys.path.insert(0, _p)

import numpy as np

import concourse.bass as bass
import concourse.mybir as mybir
import concourse.tile as tile
import bass_rust
from concourse.bass_utils import run_bass_kernel_spmd

F32 = mybir.dt.float32
AF = mybir.ActivationFunctionType
OP = mybir.AluOpType

B, N, DK = 4, 4096, 64
NQ = 2048          # queries per core
NCORES = 8
JT = N // 128      # 32 key tiles
IT = NQ // 128     # 16 query tiles
M = 512            # K-grid bins
MC = M // 128      # 4 bin chunks
K0 = -5.5
DELTA = 11.0 / (M - 1)
EPS = 1e-5

# packed const blob layout (128 partitions wide)
_IDENT0 = 0
_IOTA0 = 128
_GAM0 = 128 + M
_BET0 = _GAM0 + DK
_KAP0 = _BET0 + DK
BLOB_W = _KAP0 + MC


def split_multiwaits(nc):
    """Walrus in this env accepts one sem-wait per instruction; Tile emits
    several. Split extras onto preceding same-engine NoOps."""
    ctr = 0
    for f in nc.m.functions:
        for bb in f.blocks:
            out, changed = [], False
            for ins in bb.instructions:
                si = ins.sync_info
                if si is not None and si.on_wait and len(si.on_wait) > 1:
                    waits = list(si.on_wait)
                    for w in waits[:-1]:
                        ctr += 1
                        out.append(mybir.InstNoOp(
                            name=f"I-wsplit-{ctr}", engine=ins.engine,
                            debug=ins.debug, ins=[], outs=[],
                            sync_info=bass_rust.SyncInfo(on_wait=[w], on_update=[])))
                    ins.sync_info = bass_rust.SyncInfo(
                        on_wait=[waits[-1]], on_update=list(si.on_update or []))
                    changed = True
                out.append(ins)
            if changed:
                bb.instructions = out
    return ctr


def build_nc(split=True):
    nc = bass.Bass("TRN2", target_bir_lowering=False, debug=False)

    xr_d = nc.dram_tensor("xr", [N, DK], F32, kind="ExternalInput").ap()
    xrT_d = nc.dram_tensor("xrT", [DK, N], F32, kind="ExternalInput").ap()
    wvkb_d = nc.dram_tensor("wvkb", [DK + 1, 66], F32, kind="ExternalInput").ap()
    blob_d = nc.dram_tensor("blob", [128, BLOB_W], F32, kind="ExternalInput").ap()
    out_d = nc.dram_tensor("out", [NQ, DK], F32, kind="ExternalOutput").ap()
    qs_d = nc.dram_tensor("qscr", [1, NQ], F32).ap()        # internal scratch

    with tile.TileContext(nc) as tc:
        cpool = tc.alloc_tile_pool(name="consts", bufs=1)
        big = tc.alloc_tile_pool(name="big", bufs=1)

        blob = cpool.tile([128, BLOB_W], F32)
        nc.sync.dma_start(blob[:], blob_d[:])
        ident = blob[:, _IDENT0:_IDENT0 + 128]
        iota = blob[:, _IOTA0:_IOTA0 + M]
        gam = blob[:, _GAM0:_GAM0 + DK]
        bet = blob[:, _BET0:_BET0 + DK]
        kap = blob[:, _KAP0:_KAP0 + MC]

        wvkb = cpool.tile([DK + 1, 66], F32)
        nc.sync.dma_start(wvkb[:], wvkb_d[:])
        eps_c = cpool.tile([128, 1], F32)
        nc.gpsimd.memset(eps_c[:], EPS)

        xr_all = big.tile([128, JT * DK], F32)       # natural x, tile jt at cols jt*64
        xr_v = xr_all.rearrange("p (t d) -> p t d", d=DK)
        nc.sync.dma_start(xr_v[:], xr_d.rearrange("(t p) d -> p t d", p=128))

        xTb = big.tile([DK + 1, N], F32)             # [x.T ; ones]
        nc.sync.dma_start(xTb[0:DK, :], xrT_d[:])
        nc.gpsimd.memset(xTb[DK:DK + 1, :], 1.0)

        vaug = big.tile([128, JT * 65], F32)         # [V+bv | 1] per key tile
        vaug_v = vaug.rearrange("p (t c) -> p t c", c=65)
        tcol = big.tile([128, 2 * JT], F32)          # hat bias scratch
        q_sb = big.tile([1, NQ], F32)
        q_rep = big.tile([128, NQ], F32)             # Q replicated across partitions
        e_full = big.tile([128, MC * NQ], F32)       # exp scores per bin chunk
        e_v = e_full.rearrange("p (t i) -> p t i", i=NQ)

        with tc.tile_pool(name="prep_ps", bufs=2, space="PSUM") as pps:
            # Q row, then replicate via DRAM round-trip broadcast
            for ic in range(NQ // 512):
                qp = pps.tile([1, 512], F32, tag="qp")
                nc.tensor.matmul(qp[:], wvkb[:, 65:66],
                                 xTb[:, ic * 512:(ic + 1) * 512], start=True, stop=True)
                nc.vector.tensor_copy(q_sb[0:1, ic * 512:(ic + 1) * 512], qp[:])
            nc.sync.dma_start(qs_d[:], q_sb[:])
            nc.sync.dma_start(q_rep[:], qs_d.broadcast_to([128, NQ]))

            # [V+bv | K] per key tile; hat bias columns
            for jt in range(JT):
                vk = pps.tile([128, 65], F32, tag="vk")
                nc.tensor.matmul(vk[:], xTb[:, jt * 128:(jt + 1) * 128],
                                 wvkb[:, 0:65], start=True, stop=True)
                nc.vector.tensor_copy(vaug_v[:, jt, 0:DK], vk[:, 0:DK])
                nc.gpsimd.memset(vaug_v[:, jt, DK:65], 1.0)
                # tn = -clamp((K - K0)/DELTA, 0, M-1)
                nc.vector.tensor_scalar(tcol[:, 2 * jt:2 * jt + 1], vk[:, DK:65],
                                        -1.0 / DELTA, K0 / DELTA, OP.mult, OP.add)
                nc.vector.tensor_scalar(tcol[:, 2 * jt + 1:2 * jt + 2],
                                        tcol[:, 2 * jt:2 * jt + 1],
                                        -float(M - 1), 0.0, OP.max, OP.min)

        # ---- binning + interleaved score passes ----
        bva = big.tile([128, MC * 65], F32)          # bin-major [V|count]
        bva_v = bva.rearrange("p (t c) -> p t c", c=65)

        def emit_e_chunk(mc, ep_):
            sq = ep_.tile([128, NQ], F32, tag="sq")
            nc.scalar.activation(sq[:], q_rep[:], AF.Square,
                                 bias=kap[:, mc:mc + 1], scale=-1.0)
            a_t = ep_.tile([128, NQ], F32, tag="a")
            nc.scalar.activation(a_t[:], sq[:], AF.Exp, scale=-1.0)
            nc.scalar.activation(e_v[:, mc, :], a_t[:], AF.Exp, scale=0.125)

        with tc.tile_pool(name="acc_ps", bufs=1, space="PSUM") as accp:
            accT = accp.tile([65, NQ], F32)          # 4 banks
            with tc.tile_pool(name="bvt_ps", bufs=1, space="PSUM") as bvp:
                bvt = bvp.tile([65, M], F32)         # 1 bank
                with (tc.tile_pool(name="w_sb", bufs=3) as wp,
                      tc.tile_pool(name="e_scr", bufs=2) as ep_):
                    for jt in range(JT):
                        u_t = wp.tile([128, M], F32, tag="u")
                        nc.scalar.activation(u_t[:], iota, AF.Abs,
                                             bias=tcol[:, 2 * jt + 1:2 * jt + 2],
                                             scale=1.0)
                        w_t = wp.tile([128, M], F32, tag="w")
                        nc.vector.tensor_scalar(w_t[:], u_t[:], -1.0, 1.0,
                                                OP.mult, OP.add)
                        eng = nc.gpsimd if (jt % 2 == 0) else nc.vector
                        eng.tensor_scalar(w_t[:], w_t[:], 0.0, None, OP.max)
                        nc.tensor.matmul(bvt[:], vaug_v[:, jt, :], w_t[:],
                                         start=(jt == 0), stop=(jt == JT - 1))
                        if jt % 8 == 7:
                            emit_e_chunk(jt // 8, ep_)
                bvt_sb = big.tile([65, M], F32)
                nc.vector.tensor_copy(bvt_sb[:], bvt[:])

            with tc.tile_pool(name="tr_ps", bufs=2, space="PSUM") as trp:
                for mc in range(MC):
                    tb = trp.tile([128, 65], F32, tag="tb")
                    nc.tensor.transpose(tb[:], bvt_sb[:, mc * 128:(mc + 1) * 128],
                                        ident[0:65, 0:65])
                    nc.vector.tensor_copy(bva_v[:, mc, :], tb[:])

            for mc in range(MC):
                for c in range(NQ // 512):
                    nc.tensor.matmul(accT[:, c * 512:(c + 1) * 512],
                                     bva_v[:, mc, :],
                                     e_v[:, mc, c * 512:(c + 1) * 512],
                                     start=(mc == 0), stop=(mc == MC - 1))
            outT = big.tile([65, NQ], F32)
            nc.vector.tensor_copy(outT[:], accT[:])

        # ---- finish ----
        with tc.tile_pool(name="fin_ps", bufs=3, space="PSUM") as finp:
            nat = big.tile([128, IT * 65], F32)
            nat_v = nat.rearrange("p (t c) -> p t c", c=65)
            for it in range(IT):
                np_t = finp.tile([128, 65], F32, tag="nat")
                nc.tensor.transpose(np_t[:], outT[:, it * 128:(it + 1) * 128],
                                    ident[0:65, 0:65])
                nc.vector.tensor_copy(nat_v[:, it, :], np_t[:])

        fin = big.tile([128, IT * DK], F32)
        fin_v = fin.rearrange("p (t d) -> p t d", d=DK)
        rec = big.tile([128, IT], F32)
        stat = big.tile([128, 4 * IT], F32)
        sum_ = stat[:, 0:IT]
        m_ = stat[:, IT:2 * IT]
        v_ = stat[:, 2 * IT:3 * IT]
        rstd = stat[:, 3 * IT:4 * IT]
        scr = big.tile([128, IT * DK], F32)
        scr_v = scr.rearrange("p (t d) -> p t d", d=DK)

        nc.vector.reciprocal(rec[:], nat_v[:, :, 64])
        rec_b = rec.unsqueeze(-1).broadcast_to([128, IT, DK])
        nc.vector.tensor_tensor(fin_v[:], nat_v[:, :, 0:DK], rec_b, OP.mult)
        nc.vector.tensor_tensor(fin_v[:], fin_v[:], xr_v[:, 0:IT, :], OP.add)
        nc.vector.reduce_sum(sum_, fin_v[:], axis=mybir.AxisListType.X)
        nc.vector.tensor_scalar_mul(m_, sum_, 1.0 / DK)
        nc.vector.tensor_tensor(fin_v[:], fin_v[:],
                                m_.unsqueeze(-1).broadcast_to([128, IT, DK]), OP.subtract)
        nc.vector.tensor_tensor(scr_v[:], fin_v[:], fin_v[:], OP.mult)
        nc.vector.reduce_sum(v_, scr_v[:], axis=mybir.AxisListType.X)
        nc.scalar.activation(rstd, v_, AF.Ln, bias=eps_c[:], scale=1.0 / DK)
        nc.scalar.activation(rstd, rstd, AF.Exp, scale=-0.5)
        nc.vector.tensor_tensor(fin_v[:], fin_v[:],
                                rstd.unsqueeze(-1).broadcast_to([128, IT, DK]), OP.mult)
        nc.vector.tensor_tensor(fin_v[:], fin_v[:],
                                gam.unsqueeze(1).broadcast_to([128, IT, DK]), OP.mult)
        nc.vector.tensor_tensor(fin_v[:], fin_v[:],
                                bet.unsqueeze(1).broadcast_to([128, IT, DK]), OP.add)

        nc.sync.dma_start(out_d.rearrange("(t p) d -> p t d", p=128), fin_v[:])

        big.release()
        cpool.release()

    if split:
        split_multiwaits(nc)
    return nc


_NC_CACHE = None


def _get_nc():
    global _NC_CACHE
    if _NC_CACHE is None:
        _NC_CACHE = build_nc()
    return _NC_CACHE


def make_in_maps(x, Wv, bv, wq, wk, gamma, beta):
    x = np.asarray(x, np.float32)
    wvk = np.concatenate([np.asarray(Wv, np.float32).T,
                          np.asarray(wk, np.float32)[:, None],
                          np.asarray(wq, np.float32)[:, None]], axis=1)
    brow = np.concatenate([np.asarray(bv, np.float32), [0.0, 0.0]]).astype(np.float32)
    wvkb = np.concatenate([wvk, brow[None, :]], axis=0).copy()      # (65, 66)

    blob = np.zeros((128, BLOB_W), np.float32)
    blob[:, _IDENT0:_IDENT0 + 128] = np.eye(128, dtype=np.float32)
    blob[:, _IOTA0:_IOTA0 + M] = np.arange(M, dtype=np.float32)[None, :]
    blob[:, _GAM0:_GAM0 + DK] = np.asarray(gamma, np.float32)[None, :]
    blob[:, _BET0:_BET0 + DK] = np.asarray(beta, np.float32)[None, :]
    kgrid = (K0 + DELTA * np.arange(M, dtype=np.float64)).astype(np.float32)
    blob[:, _KAP0:_KAP0 + MC] = kgrid.reshape(MC, 128).T

    in_maps = []
    for c in range(NCORES):
        b, qoff = c // 2, (c % 2) * NQ
        xr = np.concatenate([x[b, qoff:], x[b, :qoff]], axis=0) if qoff else x[b]
        in_maps.append({"xr": np.ascontiguousarray(xr),
                        "xrT": np.ascontiguousarray(xr.T),
                        "wvkb": wvkb, "blob": blob})
    return in_maps


def kernel(x, Wv, bv, wq, wk, gamma, beta, _trace=False, _trace_cores=None):
    nc = _get_nc()
    in_maps = make_in_maps(x, Wv, bv, wq, wk, gamma, beta)
    res = run_bass_kernel_spmd(nc, in_maps, core_ids=list(range(NCORES)),
                               trace=_trace, trace_cores=_trace_cores)
    out = np.empty((B, N, DK), np.float32)
    for c in range(NCORES):
        b, qoff = c // 2, (c % 2) * NQ
        out[b, qoff:qoff + NQ] = res.results[c]["out"]
    kernel._last_results = res
    return out


# revision 26
# speedup vs baseline: 1.8620x; 1.8620x over previous
"""Trainium2 Bass kernel for nn_AttentionHead_Hybrid2 (B=4, N=4096, DK=64).

reference:
    V = x @ Wv.T + bv              (B,N,DK)
    Q = x @ wq ; K = x @ wk        (B,N)
    A = exp(-(Q_i - K_j)^2)        (B,N,N)
    P = softmax(A / 8, axis=-1)
    out = LN(P @ V + x)

Sharding: 8 cores = (batch b = c//2) x (query half c%2). Each core gets the
full key/value set for its batch (rolled so its 2048 queries are rows 0:2048)
and produces its 2048x64 output slice.

Key idea: the score between query i and key j depends on j ONLY through the
scalar K_j. Keys are binned onto a uniform 512-point grid over K-space with
linear (hat-function) interpolation, which is exact to O(delta^2):
    e(Q_i, K_j) ~= sum_m w_jm * e(Q_i, kappa_m),   w = hat((K_j-kappa_m)/delta)
so
    out_i = sum_j e_ij Vaug_j = sum_m E(Q_i, kappa_m) * (sum_j w_jm Vaug_j)
collapsing the (2048 x 4096) score work to (2048 x 512) plus cheap binning.
The hat errors are oscillatory in j and average out; measured end-to-end
error stays at fp32 noise (~1e-6 relative).

Per-core phases:
    prep:   xTb = [x.T ; 1] via DMA (host supplies x.T); [V+bv | K] = xTb.T @
            [Wv.T|wk ; bv|0]; Q row; q_rep via DRAM-broadcast DMA
    bin:    per key-tile jt: t = -clamp((K - k0)/d);  u = |m + t|  (ACT Abs)
            w = relu(1 - u)  (DVE + GpSimd);  bvt[65, m] += Vaug_jt.T @ W (PE)
    score:  per bin chunk (128 bins): E = exp(exp(-(kappa-Q)^2)/8) (3 ACT
            passes, interleaved with bin phase); accT[65,i] += bva_mc.T @ E
    finish: transpose accT, divide by rowsum (col 64), + x, LayerNorm, DMA.
"""

import sys

for _p in ("/opt/trn_rl_repo", "/root/.axon_site/_ro/trn_rl_repo"):
    if _p not in sys.path:
        sys.path.insert(0, _p)

import numpy as np

import concourse.bass as bass
import concourse.mybir as mybir
import concourse.tile as tile
import bass_rust
from concourse.bass_utils import run_bass_kernel_spmd

F32 = mybir.dt.float32
AF = mybir.ActivationFunctionType
OP = mybir.AluOpType

B, N, DK = 4, 4096, 64
NQ = 2048          # queries per core
NCORES = 8
JT = N // 128      # 32 key tiles
IT = NQ // 128     # 16 query tiles
M = 512            # K-grid bins
MC = M // 128      # 4 bin chunks
K0 = -5.5
DELTA = 11.0 / (M - 1)
EPS = 1e-5

# packed const blob layout (128 partitions wide)
_IDENT0 = 0
_IOTA0 = 128
_GAM0 = 128 + M
_BET0 = _GAM0 + DK
_KAP0 = _BET0 + DK
BLOB_W = _KAP0 + MC


def split_multiwaits(nc):
    """Walrus in this env accepts one sem-wait per instruction; Tile emits
    several. Split extras onto preceding same-engine NoOps."""
    ctr = 0
    for f in nc.m.functions:
        for bb in f.blocks:
            out, changed = [], False
            for ins in bb.instructions:
                si = ins.sync_info
                if si is not None and si.on_wait and len(si.on_wait) > 1:
                    waits = list(si.on_wait)
                    for w in waits[:-1]:
                        ctr += 1
                        out.append(mybir.InstNoOp(
                            name=f"I-wsplit-{ctr}", engine=ins.engine,
                            debug=ins.debug, ins=[], outs=[],
                            sync_info=bass_rust.SyncInfo(on_wait=[w], on_update=[])))
                    ins.sync_info = bass_rust.SyncInfo(
                        on_wait=[waits[-1]], on_update=list(si.on_update or []))
                    changed = True
                out.append(ins)
            if changed:
                bb.instructions = out
    return ctr


def build_nc(split=True):
    nc = bass.Bass("TRN2", target_bir_lowering=False, debug=False)

    xr_d = nc.dram_tensor("xr", [N, DK], F32, kind="ExternalInput").ap()
    xrT_d = nc.dram_tensor("xrT", [DK, N], F32, kind="ExternalInput").ap()
    wvkb_d = nc.dram_tensor("wvkb", [DK + 1, 66], F32, kind="ExternalInput").ap()
    blob_d = nc.dram_tensor("blob", [128, BLOB_W], F32, kind="ExternalInput").ap()
    out_d = nc.dram_tensor("out", [NQ, DK], F32, kind="ExternalOutput").ap()
    qs_d = nc.dram_tensor("qscr", [1, NQ], F32).ap()        # internal scratch

    with tile.TileContext(nc) as tc:
        cpool = tc.alloc_tile_pool(name="consts", bufs=1)
        big = tc.alloc_tile_pool(name="big", bufs=1)

        blob = cpool.tile([128, BLOB_W], F32)
        nc.sync.dma_start(blob[:], blob_d[:])
        ident = blob[:, _IDENT0:_IDENT0 + 128]
        iota = blob[:, _IOTA0:_IOTA0 + M]
        gam = blob[:, _GAM0:_GAM0 + DK]
        bet = blob[:, _BET0:_BET0 + DK]
        kap = blob[:, _KAP0:_KAP0 + MC]

        wvkb = cpool.tile([DK + 1, 66], F32)
        nc.sync.dma_start(wvkb[:], wvkb_d[:])
        eps_c = cpool.tile([128, 1], F32)
        nc.gpsimd.memset(eps_c[:], EPS)

        xr_all = big.tile([128, JT * DK], F32)       # natural x, tile jt at cols jt*64
        xr_v = xr_all.rearrange("p (t d) -> p t d", d=DK)
        nc.sync.dma_start(xr_v[:], xr_d.rearrange("(t p) d -> p t d", p=128))

        xTb = big.tile([DK + 1, N], F32)             # [x.T ; ones]
        nc.sync.dma_start(xTb[0:DK, :], xrT_d[:])
        nc.gpsimd.memset(xTb[DK:DK + 1, :], 1.0)

        vaug = big.tile([128, JT * 65], F32)         # [V+bv | 1] per key tile
        vaug_v = vaug.rearrange("p (t c) -> p t c", c=65)
        tcol = big.tile([128, 2 * JT], F32)          # hat bias scratch
        q_sb = big.tile([1, NQ], F32)
        q_rep = big.tile([128, NQ], F32)             # Q replicated across partitions
        e_full = big.tile([128, MC * NQ], F32)       # exp scores per bin chunk
        e_v = e_full.rearrange("p (t i) -> p t i", i=NQ)

        with tc.tile_pool(name="prep_ps", bufs=2, space="PSUM") as pps:
            # Q row, then replicate via DRAM round-trip broadcast
            for ic in range(NQ // 512):
                qp = pps.tile([1, 512], F32, tag="qp")
                nc.tensor.matmul(qp[:], wvkb[:, 65:66],
                                 xTb[:, ic * 512:(ic + 1) * 512], start=True, stop=True)
                nc.vector.tensor_copy(q_sb[0:1, ic * 512:(ic + 1) * 512], qp[:])
            nc.sync.dma_start(qs_d[:], q_sb[:])
            nc.sync.dma_start(q_rep[:], qs_d.broadcast_to([128, NQ]))

            # [V+bv | K] per key tile; hat bias columns
            for jt in range(JT):
                vk = pps.tile([128, 65], F32, tag="vk")
                nc.tensor.matmul(vk[:], xTb[:, jt * 128:(jt + 1) * 128],
                                 wvkb[:, 0:65], start=True, stop=True)
                nc.vector.tensor_copy(vaug_v[:, jt, 0:DK], vk[:, 0:DK])
                nc.gpsimd.memset(vaug_v[:, jt, DK:65], 1.0)
                # tn = -clamp((K - K0)/DELTA, 0, M-1)
                nc.vector.tensor_scalar(tcol[:, 2 * jt:2 * jt + 1], vk[:, DK:65],
                                        -1.0 / DELTA, K0 / DELTA, OP.mult, OP.add)
                nc.vector.tensor_scalar(tcol[:, 2 * jt + 1:2 * jt + 2],
                                        tcol[:, 2 * jt:2 * jt + 1],
                                        -float(M - 1), 0.0, OP.max, OP.min)

        # ---- binning + interleaved score passes ----
        bva = big.tile([128, MC * 65], F32)          # bin-major [V|count]
        bva_v = bva.rearrange("p (t c) -> p t c", c=65)

        def emit_e_chunk(mc, ep_):
            sq = ep_.tile([128, NQ], F32, tag="sq")
            nc.scalar.activation(sq[:], q_rep[:], AF.Square,
                                 bias=kap[:, mc:mc + 1], scale=-1.0)
            a_t = ep_.tile([128, NQ], F32, tag="a")
            nc.scalar.activation(a_t[:], sq[:], AF.Exp, scale=-1.0)
            nc.scalar.activation(e_v[:, mc, :], a_t[:], AF.Exp, scale=0.125)

        ones_col = big.tile([128, 1], F32)
        nc.gpsimd.memset(ones_col[:], 1.0)
        with tc.tile_pool(name="acc_ps", bufs=1, space="PSUM") as accp:
            accT = accp.tile([65, NQ], F32)          # 4 banks
            with tc.tile_pool(name="bvt_ps", bufs=1, space="PSUM") as bvp:
                # cols 0..511: sum_j min(u,1)*Vaug; col 512: sum_j Vaug
                bvt = bvp.tile([65, 516], F32)       # 2 banks
                with (tc.tile_pool(name="w_sb", bufs=4) as wp,
                      tc.tile_pool(name="e_scr", bufs=2) as ep_):
                    for jt in range(JT):
                        u_t = wp.tile([128, M], F32, tag="u")
                        nc.scalar.activation(u_t[:], iota, AF.Abs,
                                             bias=tcol[:, 2 * jt + 1:2 * jt + 2],
                                             scale=1.0)
                        w_t = wp.tile([128, M], F32, tag="w")
                        nc.vector.tensor_scalar(w_t[:], u_t[:], 1.0, None, OP.min)
                        nc.tensor.matmul(bvt[:, 0:M], vaug_v[:, jt, :], w_t[:],
                                         start=(jt == 0), stop=(jt == JT - 1))
                        nc.tensor.matmul(bvt[:, M:M + 1], vaug_v[:, jt, :],
                                         ones_col[:], start=(jt == 0),
                                         stop=(jt == JT - 1))
                        if jt % 8 == 7:
                            emit_e_chunk(jt // 8, ep_)
                bvt_sb = big.tile([65, M + 1], F32)
                nc.vector.tensor_copy(bvt_sb[:], bvt[:, 0:M + 1])

            # bvt_final = colsum - bvt_min  (hat = 1 - min(u,1))
            bvt_f = big.tile([65, M], F32)
            nc.vector.tensor_tensor(
                bvt_f[:], bvt_sb[:, M:M + 1].broadcast_to([65, M]),
                bvt_sb[:, 0:M], OP.subtract)

            with tc.tile_pool(name="tr_ps", bufs=2, space="PSUM") as trp:
                for mc in range(MC):
                    tb = trp.tile([128, 65], F32, tag="tb")
                    nc.tensor.transpose(tb[:], bvt_f[:, mc * 128:(mc + 1) * 128],
                                        ident[0:65, 0:65])
                    nc.vector.tensor_copy(bva_v[:, mc, :], tb[:])

            for mc in range(MC):
                for c in range(NQ // 512):
                    nc.tensor.matmul(accT[:, c * 512:(c + 1) * 512],
                                     bva_v[:, mc, :],
                                     e_v[:, mc, c * 512:(c + 1) * 512],
                                     start=(mc == 0), stop=(mc == MC - 1))
            outT = big.tile([65, NQ], F32)
            nc.vector.tensor_copy(outT[:], accT[:])

        # ---- finish ----
        with tc.tile_pool(name="fin_ps", bufs=3, space="PSUM") as finp:
            nat = big.tile([128, IT * 65], F32)
            nat_v = nat.rearrange("p (t c) -> p t c", c=65)
            for it in range(IT):
                np_t = finp.tile([128, 65], F32, tag="nat")
                nc.tensor.transpose(np_t[:], outT[:, it * 128:(it + 1) * 128],
                                    ident[0:65, 0:65])
                nc.vector.tensor_copy(nat_v[:, it, :], np_t[:])

        fin = big.tile([128, IT * DK], F32)
        fin_v = fin.rearrange("p (t d) -> p t d", d=DK)
        rec = big.tile([128, IT], F32)
        stat = big.tile([128, 4 * IT], F32)
        sum_ = stat[:, 0:IT]
        m_ = stat[:, IT:2 * IT]
        v_ = stat[:, 2 * IT:3 * IT]
        rstd = stat[:, 3 * IT:4 * IT]
        scr = big.tile([128, IT * DK], F32)
        scr_v = scr.rearrange("p (t d) -> p t d", d=DK)

        nc.vector.reciprocal(rec[:], nat_v[:, :, 64])
        rec_b = rec.unsqueeze(-1).broadcast_to([128, IT, DK])
        nc.vector.tensor_tensor(fin_v[:], nat_v[:, :, 0:DK], rec_b, OP.mult)
        nc.vector.tensor_tensor(fin_v[:], fin_v[:], xr_v[:, 0:IT, :], OP.add)
        nc.vector.reduce_sum(sum_, fin_v[:], axis=mybir.AxisListType.X)
        nc.vector.tensor_scalar_mul(m_, sum_, 1.0 / DK)
        nc.vector.tensor_tensor(fin_v[:], fin_v[:],
                                m_.unsqueeze(-1).broadcast_to([128, IT, DK]), OP.subtract)
        nc.vector.tensor_tensor(scr_v[:], fin_v[:], fin_v[:], OP.mult)
        nc.vector.reduce_sum(v_, scr_v[:], axis=mybir.AxisListType.X)
        nc.scalar.activation(rstd, v_, AF.Ln, bias=eps_c[:], scale=1.0 / DK)
        nc.scalar.activation(rstd, rstd, AF.Exp, scale=-0.5)
        nc.vector.tensor_tensor(fin_v[:], fin_v[:],
                                rstd.unsqueeze(-1).broadcast_to([128, IT, DK]), OP.mult)
        nc.vector.tensor_tensor(fin_v[:], fin_v[:],
                                gam.unsqueeze(1).broadcast_to([128, IT, DK]), OP.mult)
        nc.vector.tensor_tensor(fin_v[:], fin_v[:],
                                bet.unsqueeze(1).broadcast_to([128, IT, DK]), OP.add)

        nc.sync.dma_start(out_d.rearrange("(t p) d -> p t d", p=128), fin_v[:])

        big.release()
        cpool.release()

    if split:
        split_multiwaits(nc)
    return nc


_NC_CACHE = None


def _get_nc():
    global _NC_CACHE
    if _NC_CACHE is None:
        _NC_CACHE = build_nc()
    return _NC_CACHE


def make_in_maps(x, Wv, bv, wq, wk, gamma, beta):
    x = np.asarray(x, np.float32)
    wvk = np.concatenate([np.asarray(Wv, np.float32).T,
                          np.asarray(wk, np.float32)[:, None],
                          np.asarray(wq, np.float32)[:, None]], axis=1)
    brow = np.concatenate([np.asarray(bv, np.float32), [0.0, 0.0]]).astype(np.float32)
    wvkb = np.concatenate([wvk, brow[None, :]], axis=0).copy()      # (65, 66)

    blob = np.zeros((128, BLOB_W), np.float32)
    blob[:, _IDENT0:_IDENT0 + 128] = np.eye(128, dtype=np.float32)
    blob[:, _IOTA0:_IOTA0 + M] = np.arange(M, dtype=np.float32)[None, :]
    blob[:, _GAM0:_GAM0 + DK] = np.asarray(gamma, np.float32)[None, :]
    blob[:, _BET0:_BET0 + DK] = np.asarray(beta, np.float32)[None, :]
    kgrid = (K0 + DELTA * np.arange(M, dtype=np.float64)).astype(np.float32)
    blob[:, _KAP0:_KAP0 + MC] = kgrid.reshape(MC, 128).T

    in_maps = []
    for c in range(NCORES):
        b, qoff = c // 2, (c % 2) * NQ
        xr = np.concatenate([x[b, qoff:], x[b, :qoff]], axis=0) if qoff else x[b]
        in_maps.append({"xr": np.ascontiguousarray(xr),
                        "xrT": np.ascontiguousarray(xr.T),
                        "wvkb": wvkb, "blob": blob})
    return in_maps


def kernel(x, Wv, bv, wq, wk, gamma, beta, _trace=False, _trace_cores=None):
    nc = _get_nc()
    in_maps = make_in_maps(x, Wv, bv, wq, wk, gamma, beta)
    res = run_bass_kernel_spmd(nc, in_maps, core_ids=list(range(NCORES)),
                               trace=_trace, trace_cores=_trace_cores)
    out = np.empty((B, N, DK), np.float32)
    for c in range(NCORES):
        b, qoff = c // 2, (c % 2) * NQ
        out[b, qoff:qoff + NQ] = res.results[c]["out"]
    kernel._last_results = res
    return out
